# revision 57
# baseline (speedup 1.0000x reference)
"""Angular-prototypical hard-mining loss on 8 Trainium2 cores.

Device computes ONLY the hard part: per-row max information of the
masked (cross-label) similarity matrix sim = feats @ feats.T, which is
the single O(B^2 D) object. Everything else (pos-pair sums, thresholds,
log1p, loss assembly) is exact host math over the ~130k same-label
pairs.

Layout: host sorts rows by label so same-label columns cluster near the
diagonal. Each core gets a 1024-row slab; columns are rotated by
-core*1024 so every core runs ONE uniform SPMD program. Similarities
are computed with fp8(e4m3, x16 scaled) DoubleRow matmuls (0.5
cyc/row); the same-label + self mask (-30 in sim units) is folded into
the PE via tiny one-hot "mask matmuls" accumulated into the same PSUM
bank, so no vector-engine masking is needed. Per 128-row m-tile the
slab is processed in CHUNK_TILES*512-column PSUM chunks (PSUM_BUFS-way
double buffered); each chunk is consumed by exactly one engine,
statically interleaved so both reduction engines stay saturated:

  DVE  chunk: tensor_reduce(max, XY) -> exact per-chunk max
  ACT  chunk: exp(400*s - 160) with sum-accumulator -> log-sum-exp;
              host derives max in [LSE - ln(cols)/400, LSE]

Host decodes per-row max_neg intervals (width ~0.019 + fp8 sim noise
~0.025), decides the hard-mining thresholds with margins, and computes
pos sums exactly. Rows where any decision is ambiguous, or whose
max_neg upper bound exceeds NEGMAX (where the dropped negative-LSE term
could matter), are recomputed exactly on the CPU. On the reference data
no row is ambiguous and the dropped neg term totals ~4e-6 of a ~1.9
loss.
"""
import sys
import numpy as np

sys.path.insert(0, "/opt/trn_rl_repo")

B, D, NCORES, SLAB = 8192, 256, 8, 1024
P, NT, M_TILES, N_TILES = 128, 512, 8, 16
THRESH, MARGIN, SP, SN, EPS = 0.5, 0.1, 2.0, 50.0, 1e-5

FP8_SCALE = 16.0          # feats * 16 -> fp8 e4m3
SIMSCALE = FP8_SCALE * FP8_SCALE   # psum sim units = 256 * s
BIGM = 30.0               # mask bias in sim units
ACT_K = 400.0             # LSE sharpness (in s units)
ACT_B = 160.0             # exp(ACT_K*s - ACT_B)
DELTA = 0.028             # |sim_fp8 - sim_f32| bound (measured 0.0243)
NEGMAX = 0.40             # above this max_neg ub, neg-LSE may matter -> CPU row

MASKK = 32                # one-hot rank space for the mask matmuls
STRIP = {0: (15, 0), 1: (0,), 2: (0,), 3: (0, 1), 4: (0, 1),
         5: (1,), 6: (1,), 7: (1, 2)}
COV = {0: (-512, 512), 1: (0, 512), 2: (0, 512), 3: (0, 1024),
       4: (0, 1024), 5: (512, 1024), 6: (512, 1024), 7: (512, 1536)}
# strip slot index per (m, tile)
SLOT = {}
for _m in range(M_TILES):
    for _t in STRIP[_m]:
        SLOT[(_m, _t)] = len(SLOT)
NSLOT = len(SLOT)

# chunk -> engine assignment, interleaved. ACT lane: exp+accum.
# DVE lane: tensor_reduce(max).
def _reconfig(chunk_tiles=2, act_frac=31 / 64, psum_bufs=None, pattern="AADD"):
    global CHUNK_TILES, N_CHUNKS, PSUM_BUFS, N_ACT, ASSIGN
    global DVE_SLOT, ACT_SLOT, N_DVE, LSE_W
    CHUNK_TILES = chunk_tiles
    N_CHUNKS = (N_TILES // CHUNK_TILES) * M_TILES
    PSUM_BUFS = (8 // CHUNK_TILES) if psum_bufs is None else psum_bufs
    if pattern is not None:
        ASSIGN = [pattern[g % len(pattern)] == "A" for g in range(N_CHUNKS)]
        N_ACT = sum(ASSIGN)
    else:
        N_ACT = round(act_frac * N_CHUNKS)
        ASSIGN = []  # per chunk index: True -> ACT
        for g in range(N_CHUNKS):
            ASSIGN.append(((g + 1) * N_ACT) // N_CHUNKS > (g * N_ACT) // N_CHUNKS)
    DVE_SLOT, ACT_SLOT = {}, {}
    for g, a in enumerate(ASSIGN):
        if a:
            ACT_SLOT[g] = len(ACT_SLOT)
        else:
            DVE_SLOT[g] = len(DVE_SLOT)
    N_DVE = len(DVE_SLOT)
    LSE_W = float(np.log(CHUNK_TILES * NT)) / ACT_K


_reconfig()


def _load(tc, big, ins):
    from concourse import mybir

    F32 = mybir.dt.float32
    F8 = mybir.dt.float8e4
    nc = tc.nc

    fks = big.tile([P, 2, SLAB], F8, name="fks")
    fkm = [big.tile([P, 2, 2048], F8, name=f"fkm{q}") for q in range(4)]
    mrow = big.tile([MASKK, M_TILES * P], F8, name="mrow")
    mcol = big.tile([MASKK, NSLOT * NT], F8, name="mcol")
    bias_a = big.tile([P, 1], F32, name="bias_a")
    warm = big.tile([P, 1], F32, name="warm")
    nc.vector.memset(bias_a[:], -float(ACT_B))
    # preload the Exp activation table during the DMA fill (saves its
    # 1.3us load from the first real chunk's critical path)
    from concourse.mybir import ActivationFunctionType as _Act
    nc.scalar.activation(out=warm[:], in_=bias_a[:], func=_Act.Exp,
                         scale=1.0, bias=bias_a[:])

    nc.sync.dma_start(mrow[:], ins["mrow"][:])
    nc.sync.dma_start(mcol[:], ins["mcol"][:])
    nc.gpsimd.dma_start(fks[:], ins["fks"][:])
    engs = [nc.sync, nc.gpsimd]
    for q in range(4):
        for h in range(2):
            sl = slice(h * 2 * NT, (h + 1) * 2 * NT)
            e = engs[(2 * q + h) % len(engs)]
            e.dma_start(fkm[q][:, :, sl], ins[f"fkm{q}"][:, :, sl])
    return {"fks": fks, "fkm": fkm, "mrow": mrow, "mcol": mcol,
            "bias_a": bias_a}


def _compute(tc, big, psp, tiles, outs):
    from concourse import mybir

    F32 = mybir.dt.float32
    Alu, Act = mybir.AluOpType, mybir.ActivationFunctionType
    X = mybir.AxisListType.X
    DR = mybir.MatmulPerfMode.DoubleRow
    nc = tc.nc
    fks, fkm = tiles["fks"], tiles["fkm"]
    mrow, mcol, bias_a = tiles["mrow"], tiles["mcol"], tiles["bias_a"]

    dvemax_o = big.tile([P, N_DVE], F32, tag="dvemax_o")
    actse_o = big.tile([P, N_ACT], F32, tag="actse_o")

    CT = CHUNK_TILES
    CPM = N_TILES // CT           # chunks per m-tile
    for m in range(M_TILES):
        lhs = fks[:, :, m * P:(m + 1) * P]
        for c in range(CPM):
            g = CPM * m + c
            pt = psp.tile([P, CT, NT], F32, tag="ps")
            for k in range(CT):
                t = CT * c + k
                q, qk = t // 4, t % 4
                is_strip = (m, t) in SLOT
                nc.tensor.matmul(
                    pt[:, k:k + 1, :], lhs,
                    fkm[q][:, :, qk * NT:(qk + 1) * NT],
                    start=True, stop=not is_strip, perf_mode=DR)
                if is_strip:
                    s = SLOT[(m, t)]
                    nc.tensor.matmul(
                        pt[:, k:k + 1, :],
                        mrow[:, m * P:(m + 1) * P],
                        mcol[:, s * NT:(s + 1) * NT],
                        start=False, stop=True)
            if ASSIGN[g]:
                a = ACT_SLOT[g]
                nc.scalar.activation(
                    out=pt[:], in_=pt[:], func=Act.Exp,
                    scale=float(ACT_K / SIMSCALE), bias=bias_a[:],
                    accum_out=actse_o[:, a:a + 1])
            else:
                d = DVE_SLOT[g]
                nc.vector.tensor_reduce(
                    dvemax_o[:, d:d + 1], pt[:], axis=mybir.AxisListType.XY,
                    op=Alu.max)

    nc.sync.dma_start(outs["dvemax"][:], dvemax_o[:])
    nc.sync.dma_start(outs["actse"][:], actse_o[:])


def _loss_kernel(tc, outs, ins, reps=1):
    from contextlib import ExitStack

    with ExitStack() as ctx:
        big = ctx.enter_context(tc.tile_pool(name="big", bufs=1))
        rep_pool = ctx.enter_context(tc.tile_pool(name="rep", bufs=2))
        psp = ctx.enter_context(
            tc.tile_pool(name="psum", bufs=PSUM_BUFS, space="PSUM"))
        tiles = _load(tc, big, ins)
        for _ in range(reps):
            _compute(tc, rep_pool, psp, tiles, outs)


def _numpy_fallback(feats, labels):
    f = np.float32
    sim = feats @ feats.T
    same = labels[:, None] == labels[None, :]
    pos_mask = same & (sim < f(1.0 - EPS))
    neg_mask = ~same
    min_pos = np.where(pos_mask, sim, np.inf).min(axis=1).astype(np.float32)
    max_neg = np.where(neg_mask, sim, -np.inf).max(axis=1).astype(np.float32)
    neg_sel = neg_mask & (sim > (min_pos - f(MARGIN))[:, None])
    pos_sel = pos_mask & (sim < (max_neg + f(MARGIN))[:, None])
    valid = neg_sel.any(axis=1) & pos_sel.any(axis=1)
    ps = np.exp(np.where(pos_sel, -f(SP) * (sim - f(THRESH)), -np.inf),
                dtype=np.float32).sum(axis=1, dtype=np.float32)
    ns = np.exp(np.where(neg_sel, f(SN) * (sim - f(THRESH)), -np.inf),
                dtype=np.float32).sum(axis=1, dtype=np.float32)
    rl = (f(1.0 / SP) * np.log1p(ps) + f(1.0 / SN) * np.log1p(ns)).astype(np.float32)
    loss = np.float32(np.where(valid, rl, f(0)).sum(dtype=np.float32) / f(B))
    prec1 = np.float32(np.mean((1.0 - valid.astype(np.float32)), dtype=np.float32))
    return loss, prec1


def _exact_rows(fs, labs, rows):
    """Exact reference row logic for the given sorted-row indices.
    Returns (row_loss, valid) arrays aligned with `rows`."""
    f = np.float32
    sim = fs[rows] @ fs.T
    same = labs[rows][:, None] == labs[None, :]
    pos_mask = same & (sim < f(1.0 - EPS))
    neg_mask = ~same
    min_pos = np.where(pos_mask, sim, np.inf).min(axis=1)
    max_neg = np.where(neg_mask, sim, -np.inf).max(axis=1)
    neg_sel = neg_mask & (sim > (min_pos - f(MARGIN))[:, None])
    pos_sel = pos_mask & (sim < (max_neg + f(MARGIN))[:, None])
    valid = neg_sel.any(axis=1) & pos_sel.any(axis=1)
    ps = np.exp(np.where(pos_sel, -f(SP) * (sim - f(THRESH)), -np.inf),
                dtype=np.float32).sum(axis=1, dtype=np.float32)
    ns = np.exp(np.where(neg_sel, f(SN) * (sim - f(THRESH)), -np.inf),
                dtype=np.float32).sum(axis=1, dtype=np.float32)
    rl = (f(1.0 / SP) * np.log1p(ps) + f(1.0 / SN) * np.log1p(ns)).astype(np.float32)
    return rl, valid


def _prepare(feats, labels):
    """Sort by label, quantize, build per-core device inputs.
    Returns (ins_list, out_like, ctx) or None if layout assumptions fail."""
    import ml_dtypes

    feats = np.ascontiguousarray(np.asarray(feats), dtype=np.float32)
    labels = np.asarray(labels).astype(np.int64).ravel()
    perm = np.argsort(labels, kind="stable")
    labs = labels[perm]
    fs = feats[perm]

    nlab = int(labs.max()) + 1 if labs.size else 1
    counts = np.bincount(labs, minlength=nlab)
    starts = np.cumsum(counts) - counts
    gs_row = starts[labs]
    ge_row = (starts + counts)[labs]
    for c in range(NCORES):
        base = c * SLAB
        for m in range(M_TILES):
            r = slice(base + m * P, base + (m + 1) * P)
            lo, hi = COV[m]
            if (gs_row[r] - base < lo).any() or (ge_row[r] - base > hi).any():
                return None

    F8NP = ml_dtypes.float8_e4m3
    fq8 = (fs * np.float32(FP8_SCALE)).astype(F8NP)           # [B, D]
    fqT = np.ascontiguousarray(fq8.T)                          # [D, B] fp8

    ins_list = []
    for c in range(NCORES):
        rotT = np.roll(fqT, -c * SLAB, axis=1)                 # [256, B]
        lab_rot = np.roll(labs, -c * SLAB)
        arr = rotT.reshape(2, P, B).swapaxes(0, 1)             # [128, 2, B]
        ins = {
            "fks": np.ascontiguousarray(arr[:, :, :SLAB]),
        }
        for q in range(4):
            ins[f"fkm{q}"] = np.ascontiguousarray(arr[:, :, q * 2048:(q + 1) * 2048])
        mrow = np.zeros((MASKK, M_TILES * P), F8NP)
        mcol = np.zeros((MASKK, NSLOT * NT), F8NP)
        for m in range(M_TILES):
            rl = labs[c * SLAB + m * P: c * SLAB + (m + 1) * P]
            uniq, ranks = np.unique(rl, return_inverse=True)
            if len(uniq) > MASKK:
                return None
            mrow[ranks, m * P + np.arange(P)] = F8NP(64.0)
            lut = {int(v): k for k, v in enumerate(uniq)}
            for t in STRIP[m]:
                s = SLOT[(m, t)]
                cl = lab_rot[t * NT:(t + 1) * NT]
                for j, lv in enumerate(cl):
                    k = lut.get(int(lv))
                    if k is not None:
                        mcol[k, s * NT + j] = F8NP(-120.0)
        ins["mrow"] = mrow
        ins["mcol"] = mcol
        ins_list.append(ins)

    out_like = {"dvemax": np.zeros((P, N_DVE), np.float32),
                "actse": np.zeros((P, N_ACT), np.float32)}
    ctx = {"fs": fs, "labs": labs, "perm": perm}
    return ins_list, out_like, ctx


def _decode(core_results, ctx):
    """Host decode: per-row max_neg interval -> thresholds -> exact pos
    sums; exact CPU recompute for ambiguous/hot rows."""
    f = np.float32
    fs, labs = ctx["fs"], ctx["labs"]

    # per-row max interval (in true-sim units)
    max_lb = np.full(B, -np.inf, np.float64)
    max_ub = np.full(B, -np.inf, np.float64)
    bad = np.zeros(B, bool)
    CT = CHUNK_TILES
    CPM = N_TILES // CT
    for c in range(NCORES):
        dm = np.asarray(core_results[c]["dvemax"], np.float64)   # [128, CT*N_DVE]
        se = np.asarray(core_results[c]["actse"], np.float64)    # [128, N_ACT]
        rows = c * SLAB + np.arange(SLAB).reshape(M_TILES, P)
        for g in range(N_CHUNKS):
            m = g // CPM
            r = rows[m]
            if ASSIGN[g]:
                a = ACT_SLOT[g]
                v = se[:, a]
                okv = np.isfinite(v)
                bad[r] |= ~okv
                vv = np.where(okv & (v > 0), v, 1e-300)
                ub = (np.log(vv) + ACT_B) / ACT_K
                ub = np.where(okv & (v > 0), ub, (-87.0 + ACT_B) / ACT_K)
                lb = np.where(okv & (v > 0), ub - LSE_W, -np.inf)
                max_ub[r] = np.maximum(max_ub[r], ub)
                max_lb[r] = np.maximum(max_lb[r], lb)
            else:
                d = DVE_SLOT[g]
                mx = dm[:, d] / SIMSCALE
                max_ub[r] = np.maximum(max_ub[r], mx)
                max_lb[r] = np.maximum(max_lb[r], mx)
    max_lb = max_lb - DELTA
    max_ub = max_ub + DELTA

    # exact pos-pair pass per label group
    nlab = int(labs.max()) + 1
    counts = np.bincount(labs, minlength=nlab)
    starts = np.cumsum(counts) - counts
    min_pos = np.full(B, np.inf, np.float32)
    pos_sum = np.zeros(B, np.float64)
    ambig = np.zeros(B, bool)
    tp_lo = (max_lb + MARGIN).astype(np.float32)
    tp_hi = (max_ub + MARGIN).astype(np.float32)
    for lv in range(nlab):
        n = counts[lv]
        if n == 0:
            continue
        r0 = starts[lv]
        idx = np.arange(r0, r0 + n)
        if n == 1:
            continue
        G = (fs[idx] @ fs[idx].T).astype(np.float32)
        pm = (~np.eye(n, dtype=bool)) & (G < f(1.0 - EPS))
        min_pos[idx] = np.where(pm, G, np.inf).min(1)
        lo = tp_lo[idx][:, None]
        hi = tp_hi[idx][:, None]
        ambig[idx] |= (pm & (G >= lo) & (G <= hi)).any(1)
        sel = pm & (G < lo)
        pos_sum[idx] = np.exp(np.where(sel, -SP * (G.astype(np.float64) - THRESH),
                                       -np.inf)).sum(1)

    # validity decisions with margins
    thr_n = min_pos - f(MARGIN)          # need max_neg > thr_n
    vneg_yes = max_lb > thr_n
    vneg_no = max_ub < thr_n
    vpos_yes = min_pos < tp_lo
    vpos_no = min_pos > tp_hi
    ambig |= ~(vneg_yes | vneg_no) | ~(vpos_yes | vpos_no)
    ambig |= bad
    ambig |= max_ub > NEGMAX             # dropped neg-LSE might matter

    valid = vneg_yes & vpos_yes
    row_loss = np.where(valid, f(1.0 / SP) * np.log1p(pos_sum), 0.0)

    n_amb = int(ambig.sum())
    if n_amb > 2048:
        return None
    if n_amb:
        rows = np.nonzero(ambig)[0]
        rl, vd = _exact_rows(fs, labs, rows)
        row_loss[rows] = np.where(vd, rl, 0.0)
        valid[rows] = vd

    loss = np.float32(row_loss.sum() / B)
    prec1 = np.float32(np.mean(1.0 - valid.astype(np.float32)))
    return loss, prec1


def kernel(feats, labels):
    feats = np.ascontiguousarray(np.asarray(feats), dtype=np.float32)
    labels = np.asarray(labels).astype(np.int64).ravel()

    prep = _prepare(feats, labels)
    if prep is None:
        return _numpy_fallback(feats, labels)
    ins_list, out_like, ctx = prep

    from concourse.bass_test_utils import run_kernel
    import concourse.tile as tile

    res = run_kernel(
        _loss_kernel, None, ins_list, output_like=[out_like] * NCORES,
        bass_type=tile.TileContext, num_cores=NCORES,
        check_with_sim=False, check_with_hw=True, trace_sim=False,
        trace_hw=False,
    )

    def grab(cr, key):
        for k, v in cr.items():
            if key in k:
                return np.asarray(v)
        raise KeyError(key)

    core_results = [{"dvemax": grab(res.results[c], "dvemax"),
                     "actse": grab(res.results[c], "actse")}
                    for c in range(NCORES)]
    out = _decode(core_results, ctx)
    if out is None:
        return _numpy_fallback(feats, labels)
    return out
